# revision 9
# baseline (speedup 1.0000x reference)
"""Trainium2 Bass kernel for nn_NeuronS3DiffUpsample2D.

Reference computation (per sample b):
    up   = nearest-2x-upsample(x[b])                       # [C, 320, 320]
    w    = Wb + 0.25 * einsum('or,rikl->oikl', lora_up, lora_down)
    w_b  = w * de_mod[b, None, :, None, None]              # modulate input chans
    dem  = rsqrt(sum_{i,k,l} w_b^2 + eps)                  # per output chan
    y[b] = conv2d(up, w_b * dem, SAME) + bias

Key algebraic transform: a 3x3 SAME conv on a 2x nearest-upsampled image
decomposes into 4 output phases (di, dj in {0,1}), each a 2x2 conv on the
ORIGINAL 160x160 input:
    y[2i+di, 2j+dj] = sum_{a,b in {0,1}} K[di,dj,a,b] @ x[i+a+di-1, j+b+dj-1]
where the 16 [O, I] matrices K are sums of 1/2/4 of the 9 taps of w.
This is 4/9 of the naive FLOPs and never materializes the upsampled image.

Since the demod scale is per output channel and conv is linear in w, the conv
OUTPUT is scaled by dem[o] (per-partition scalar) at PSUM eviction, fused with
the bias add; weights are only modulated by de_mod on the input-channel axis.

Sharding: data-parallel over batch B=8 across 8 NeuronCores; each core builds
its own per-sample weights locally (replicated W/lora are tiny).  Host-side
work is layout only: per-sample slicing, weight transposition, fp32->bf16
rounding.  All arithmetic (lora matmul, modulation, demod, conv) is on device.

Conv matmuls run in bf16: the PE streams bf16 at 1 cycle/row (same as f32r)
but LDWEIGHTS loads a 2-byte stationary matrix in ~100ns, fully hidden under
the previous matmul's 480-column (200ns) stream; with f32r weights the 225ns
LDWEIGHTS was the critical path (262ns cadence).  bf16 also halves input and
output DMA.  Demod/bias ride the f32 PSUM eviction, so accumulated precision
loss is only on the bf16 operands (measured rel err 2.9e-3 vs 2e-2 budget).
Output goes to DRAM as bf16 and is widened to f32 on the host.

Preamble scheduling: the lora DMAs are issued first (they gate the delta
matmuls), the 8 "single-tap" conv weights are read straight out of wm3/R01/
R10 slices (no copy), and the demod summation matmul is emitted AFTER conv
block 0 - the tensor queue is in-order, so putting it before the conv loop
made every conv matmul wait ~9us behind the DVE tap-sum reduce it depends on.

The input is banded into 6 SBUF tiles so conv matmuls start as soon as the
first band lands instead of waiting for the full input.
"""

import sys
import numpy as np
import ml_dtypes
from contextlib import ExitStack

try:
    import concourse.bass as bass
except ImportError:  # grading env without the axon PYTHONPATH
    sys.path.insert(0, "/opt/trn_rl_repo")
    import concourse.bass as bass
import concourse.tile as tile
from concourse import bacc, mybir
from concourse.bass_utils import run_bass_kernel_spmd

B, C, H, W = 8, 128, 160, 160
RANK = 32
SCALING = 0.25
EPS = 1e-8
WP = W + 2          # padded row length (zero col on each side)
R_BLK = 3           # x-rows per matmul block -> N = 3*160 = 480 <= 512
BAND_BLOCKS = 9     # blocks per input band
BAND_ROWS = BAND_BLOCKS * R_BLK      # 27 x-rows per band
NBANDS = (H + BAND_ROWS - 1) // BAND_ROWS   # 6
BAND_TROWS = BAND_ROWS + 2           # tile rows incl. halo (29)
NCORES = 8

f32 = mybir.dt.float32
bf16 = mybir.dt.bfloat16


def _conv_kernel(ctx, tc, y, x, dmbias, wbT, luT, ldT, ident2):
    nc = tc.nc
    AF = mybir.ActivationFunctionType
    ALU = mybir.AluOpType
    AX = mybir.AxisListType

    const = ctx.enter_context(tc.tile_pool(name="const", bufs=1))
    bands = ctx.enter_context(tc.tile_pool(name="bands", bufs=3))

    # dmbias/ident2 (tiny, gate the de_mod transpose) get the sync queue to
    # themselves; the three weight tensors ride the otherwise-idle ACT queue
    # so no transfer waits behind the 295KB WbT.  Bands are on gpsimd.
    dmbR = const.tile([2, C], f32)
    nc.sync.dma_start(dmbR[:], dmbias[:])
    id2 = const.tile([2, 2], f32)
    nc.sync.dma_start(id2[:], ident2[:])
    LUTn = const.tile([RANK, C], bf16)           # lora_up^T: [r, o]
    nc.sync.dma_start(LUTn[:], luT[:])
    LD9 = const.tile([RANK, 9, C], bf16)         # lora_down^T: [r, t, i]
    nc.sync.dma_start(LD9[:], ldT[:])
    WbTS = const.tile([128, 9, C], bf16)         # Wb^T: [i, t, o]
    nc.scalar.dma_start(WbTS[:], wbT[:])

    # weight tensors that the conv loop reads as stationary operands
    wm3 = const.tile([128, 9, C], bf16)          # modulated w^T: [i, t, o]
    R01 = const.tile([128, 3, C], bf16)          # rows ki1+ki2
    R10 = const.tile([128, 3, C], bf16)          # rows ki0+ki1
    comb8 = const.tile([128, 8, C], bf16)        # the 8 two-column tap sums
    demP = const.tile([128, 1], f32)             # rsqrt demod, per output chan
    dmb = const.tile([128, 3], f32)              # de_mod[i], bias[o], 0.25*de_mod
    s2 = const.tile([128, C], f32)               # per-(i,o) tap-summed squares
    onesS = const.tile([128, 1], f32)
    zrow = const.tile([128, WP], bf16)

    nc.vector.memset(zrow[:], 0.0)
    nc.vector.memset(onesS[:], 1.0)

    # ---- input bands.  The first 27 rows are split into three staged
    # segments (blocks 0-1 / 2-4 / 5-8) so the first conv matmul isn't
    # gated behind a 1.2MB transfer; later rows use full 27-row bands.
    # SWDGE via the otherwise-idle GpSimd engine: HWDGE descriptor
    # generation for these many-descriptor DMAs would occupy the sync
    # sequencer and stall evictions behind it.  (lo, hi) are halo-
    # inclusive x-row bounds; out-of-range rows are zeroed.
    segpool = ctx.enter_context(tc.tile_pool(name="segs", bufs=3))
    segs = [(-1, 6), (5, 15), (14, 27)] + [
        (BAND_ROWS * bb - 1, min(BAND_ROWS * bb + BAND_ROWS, H))
        for bb in range(1, NBANDS)
    ]
    band_tiles = []
    for lo, hi in segs:
        nrows = hi - lo + 1
        pool = segpool if hi <= BAND_ROWS else bands
        bt = pool.tile([128, nrows, WP], bf16, tag=f"band{lo}", name=f"band{lo}")
        r0, r1 = max(0, lo), min(H - 1, hi)          # real x rows
        nc.gpsimd.dma_start(
            bt[:, r0 - lo : r1 - lo + 1, 1 : 1 + W], x[:, r0 : r1 + 1, :]
        )
        nc.vector.tensor_copy(bt[:, 0:nrows, 0], zrow[:, 0:nrows])
        nc.vector.tensor_copy(bt[:, 0:nrows, WP - 1], zrow[:, 0:nrows])
        if lo < 0:
            nc.vector.tensor_copy(bt[:, 0, :], zrow[:])
        if hi >= H:
            nc.vector.tensor_copy(bt[:, hi - lo, :], zrow[:])
        band_tiles.append((bt, lo, hi))

    def _band_for(i0, R):
        for bt, lo, hi in band_tiles:
            if lo <= i0 - 1 and i0 + R <= hi:
                return bt, lo
        raise AssertionError(f"no band for block {i0}")

    # ---- weight stage ----
    with tc.tile_pool(name="wtmp", bufs=1) as wtmp, tc.tile_pool(
        name="wpsum", bufs=1, space="PSUM"
    ) as wpsum:
        dmbP = wpsum.tile([128, 2], f32)
        nc.tensor.transpose(dmbP[:], dmbR[:], id2[:])
        nc.vector.tensor_copy(dmb[:, 0:2], dmbP[:])
        nc.vector.tensor_scalar_mul(dmb[:, 2:3], dmb[:, 0:1], SCALING)

        # deltaT_unscaled[i, t, o] = sum_r down[r,i,t] * up[o,r]; the 0.25
        # lora scale rides in via the fused modulation below
        deltaP = wpsum.tile([128, 9, C], f32)
        for t in range(9):
            nc.tensor.matmul(
                deltaP[:, t, :], LD9[:, t, :], LUTn[:], start=True, stop=True
            )

        # wm3 = Wb^T*dm + deltaT*(0.25*dm); Wb^T*dm runs while the delta
        # matmuls are still in flight, the fused op is one DVE pass
        WbTm = wtmp.tile([128, 9, C], bf16)
        nc.vector.tensor_scalar_mul(WbTm[:], WbTS[:], dmb[:, 0:1])
        nc.vector.scalar_tensor_tensor(
            wm3[:], deltaP[:], dmb[:, 2:3], WbTm[:],
            op0=ALU.mult, op1=ALU.add,
        )

        # Row combos over ki (t = 3*ki + kj):
        #   (di=0, a=0): ki0        (di=0, a=1): ki1+ki2
        #   (di=1, a=0): ki0+ki1    (di=1, a=1): ki2
        # and the same pattern over kj for (dj, b).  The 8 "single-column"
        # taps (dj=0,b=0 -> kj0 and dj=1,b=1 -> kj2) are read directly out
        # of wm3/R01/R10 by the conv loop; only the 8 two-column sums are
        # materialized, phase-0 slots first.
        nc.vector.tensor_add(R01[:], wm3[:, 3:6, :], wm3[:, 6:9, :])
        nc.vector.tensor_add(R10[:], wm3[:, 0:3, :], wm3[:, 3:6, :])
        rowsrc = {
            (0, 0): wm3[:, 0:3, :],
            (0, 1): R01[:],
            (1, 0): R10[:],
            (1, 1): wm3[:, 6:9, :],
        }
        for p in range(4):
            di, dj = p >> 1, p & 1
            for a in range(2):
                S = rowsrc[(di, a)]
                dst = comb8[:, 4 * di + 2 * a + dj, :]
                if dj == 0:      # (dj=0, b=1): kj1 + kj2
                    nc.vector.tensor_add(dst, S[:, 1, :], S[:, 2, :])
                else:            # (dj=1, b=0): kj0 + kj1
                    nc.vector.tensor_add(dst, S[:, 0, :], S[:, 1, :])

        # demod source: sq3 = wm3^2 (ACT), tap-sum on DVE.  The partition
        # sum (a PE matmul with ones) is emitted AFTER conv block 0 so the
        # in-order tensor queue doesn't stall the conv behind this reduce.
        sq3 = wtmp.tile([128, 9, C], f32)
        nc.scalar.square(sq3[:], wm3[:])
        nc.vector.tensor_reduce(
            s2[:], sq3.rearrange("p t o -> p o t"), axis=AX.X, op=ALU.add
        )

    def _conv_weight(di, dj, a, b):
        if dj == 0 and b == 0:
            return rowsrc[(di, a)][:, 0, :]
        if dj == 1 and b == 1:
            return rowsrc[(di, a)][:, 2, :]
        return comb8[:, 4 * di + 2 * a + dj, :]

    # ---- main conv loop ----
    mpsum = ctx.enter_context(tc.tile_pool(name="mpsum", bufs=7, space="PSUM"))
    spsum = ctx.enter_context(tc.tile_pool(name="spsum", bufs=1, space="PSUM"))
    opool = ctx.enter_context(tc.tile_pool(name="obuf", bufs=3))

    for i0 in range(0, H, R_BLK):
        R = min(R_BLK, H - i0)
        bt, lo = _band_for(i0, R)
        ph = []
        for p in range(4):
            di, dj = p >> 1, p & 1
            pt = mpsum.tile([128, R * W], f32, tag="ph", name=f"ph{p}_{i0}")
            for q in range(4):
                a, b = q >> 1, q & 1
                r0 = i0 + (a + di - 1) - lo          # tile row of first x row
                co = b + dj - 1
                rhs = bt[:, r0 : r0 + R, co + 1 : co + 1 + W]
                nc.tensor.matmul(
                    pt[:], _conv_weight(di, dj, a, b), rhs,
                    start=(q == 0), stop=(q == 3),
                )
            ph.append(pt)
        if i0 == 0:
            # demod[o] = rsqrt(sum_i s2[i,o] + eps): partition sum via PE,
            # queued behind block 0's matmuls (needed only by evictions).
            sP = spsum.tile([128, 1], f32)
            nc.tensor.matmul(sP[:], s2[:], onesS[:], start=True, stop=True)
            t1 = const.tile([128, 1], f32)
            nc.vector.tensor_scalar_add(t1[:], sP[:], EPS)
            t2 = const.tile([128, 1], f32)
            nc.scalar.sqrt(t2[:], t1[:])
            nc.vector.reciprocal(demP[:], t2[:])
        # interleave phases into full output rows; scale by demod, add bias
        ob = opool.tile([128, R, 2, 2 * W], bf16, tag="ob", name=f"ob_{i0}")
        obv = ob.rearrange("p r d (j two) -> p r d two j", two=2)
        for p in range(4):
            di, dj = p >> 1, p & 1
            dst = obv[:, :, di, dj, :]
            srcv = ph[p].rearrange("p (r j) -> p r j", r=R)
            if dj == 0:
                nc.vector.tensor_scalar(
                    dst, srcv, demP[:, 0:1], dmb[:, 1:2],
                    op0=ALU.mult, op1=ALU.add,
                )
            else:
                nc.scalar.activation(
                    dst, srcv, AF.Identity, bias=dmb[:, 1:2], scale=demP[:, 0:1]
                )
        nc.sync.dma_start(y[:, 2 * i0 : 2 * i0 + 2 * R, :], ob[:])


def _build():
    nc = bacc.Bacc(
        "TRN2",
        target_bir_lowering=False,
        debug=False,
        enable_asserts=False,
        num_devices=NCORES,
    )
    x = nc.dram_tensor("x", [C, H, W], bf16, kind="ExternalInput").ap()
    dmbias = nc.dram_tensor("dmbias", [2, C], f32, kind="ExternalInput").ap()
    wbT = nc.dram_tensor("WbT", [C, 9 * C], bf16, kind="ExternalInput").ap()
    luT = nc.dram_tensor("lora_upT", [RANK, C], bf16, kind="ExternalInput").ap()
    ldT = nc.dram_tensor("lora_downT", [RANK, 9 * C], bf16, kind="ExternalInput").ap()
    ident2 = nc.dram_tensor("ident2", [2, 2], f32, kind="ExternalInput").ap()
    y = nc.dram_tensor("y", [C, 2 * H, 2 * W], bf16, kind="ExternalOutput").ap()

    with tile.TileContext(nc) as tc:
        with ExitStack() as ctx:
            _conv_kernel(ctx, tc, y, x, dmbias, wbT, luT, ldT, ident2)
    nc.compile()
    return nc


_CACHE = {}


def _get_nc():
    if "nc" not in _CACHE:
        _CACHE["nc"] = _build()
    return _CACHE["nc"]


def _make_in_maps(x, de_mod, Wb, lora_up, lora_down, bias):
    bf = ml_dtypes.bfloat16
    x = np.asarray(x, dtype=np.float32).astype(bf)
    de_mod = np.asarray(de_mod, dtype=np.float32)
    Wb = np.asarray(Wb, dtype=np.float32)
    lora_up = np.asarray(lora_up, dtype=np.float32)
    lora_down = np.asarray(lora_down, dtype=np.float32)
    # layout-only host prep: [O,I,3,3] -> [i, (t o)], [R,C,3,3] -> [r, (t i)]
    wbT = np.ascontiguousarray(Wb.transpose(1, 2, 3, 0).reshape(C, 9 * C)).astype(bf)
    luT = np.ascontiguousarray(lora_up.T).astype(bf)
    ldT = np.ascontiguousarray(
        lora_down.transpose(0, 2, 3, 1).reshape(RANK, 9 * C)
    ).astype(bf)
    bias = np.asarray(bias, dtype=np.float32).reshape(C)
    id2 = np.eye(2, dtype=np.float32)
    in_maps = []
    for b in range(NCORES):
        in_maps.append(
            {
                "x": np.ascontiguousarray(x[b]),
                "dmbias": np.ascontiguousarray(np.stack([de_mod[b], bias])),
                "WbT": wbT,
                "lora_upT": luT,
                "lora_downT": ldT,
                "ident2": id2,
            }
        )
    return in_maps


def run(inputs, trace=False, trace_kwargs=None):
    nc = _get_nc()
    in_maps = _make_in_maps(**inputs)
    res = run_bass_kernel_spmd(
        nc,
        in_maps,
        core_ids=list(range(NCORES)),
        trace=trace,
        **(trace_kwargs or {}),
    )
    y = np.stack(
        [res.results[b]["y"].astype(np.float32) for b in range(NCORES)], axis=0
    )
    return y, res


def kernel(**inputs):
    y, _ = run(inputs)
    return y
